# revision 5
# baseline (speedup 1.0000x reference)
import sys

if "/opt/trn_rl_repo" not in sys.path:
    sys.path.insert(0, "/opt/trn_rl_repo")

import numpy as np
import ml_dtypes

from concourse import bass, tile, bacc
from concourse.bass import mybir

F32 = mybir.dt.float32
BF16 = mybir.dt.bfloat16

N_CORES = 8
N_TOTAL = 32768
N_CORE = N_TOTAL // N_CORES  # 4096 rows per core
D = 1024
C = 64
K = 16
DEPTH = 4
M = 1024
N_STAGES = 2
ROWS_STAGE = N_CORE // N_STAGES  # 2048
ALU = mybir.AluOpType
AFT = mybir.ActivationFunctionType


def build_program(dims=None, repeat=1):
    """dims kept for signature compat (gather now happens on host)."""
    nc = bacc.Bacc()
    xg_d = nc.declare_dram_parameter(
        "xg", [N_STAGES, 128, DEPTH, ROWS_STAGE // 2], F32, isOutput=False
    )
    thr_d = nc.declare_dram_parameter("thrcols", [128, 15], F32, isOutput=False)
    lut_d = nc.declare_dram_parameter("lutT", [C * K, M], BF16, isOutput=False)
    kvec_d = nc.declare_dram_parameter("kvec", [128, 1], F32, isOutput=False)
    out_d = nc.declare_dram_parameter("out", [N_CORE, M], BF16, isOutput=True)

    NH = ROWS_STAGE // 2  # 1024 rows per partition-half

    with tile.TileContext(nc) as tc:
        from contextlib import ExitStack
        es = ExitStack()
        pers = es.enter_context(tc.tile_pool(name="pers", bufs=1))

        def ptile(shape, dtype, name):
            return pers.tile(shape, dtype, name=name, tag=name)

        # ---- persistent tiles ----
        lutT = ptile([128, 8, M], BF16, "lutT_sb")       # [j*128+p row, m]
        thr = ptile([128, 15], F32, "thr_sb")
        kvec = ptile([128, 1], F32, "kvec_sb")
        tmps = [ptile([128, NH], F32, f"tmp{ti}_sb") for ti in range(10)]
        b0, b1, b2, tA, tB, tC, tD, tE, tF, tG = tmps
        I8 = mybir.dt.int8
        b0i = ptile([128, NH], I8, "b0i_sb")
        b1i = ptile([128, NH], I8, "b1i_sb")

        xpool = es.enter_context(tc.tile_pool(name="xpool", bufs=2))
        bpool = es.enter_context(tc.tile_pool(name="bpool", bufs=2))
        epool = es.enter_context(tc.tile_pool(name="epool", bufs=2))
        opool = es.enter_context(tc.tile_pool(name="opool", bufs=2))
        pspool = es.enter_context(
            tc.tile_pool(name="pspool", bufs=2, space=bass.MemorySpace.PSUM)
        )

        nc.sync.dma_start(thr[:], thr_d[:])
        nc.sync.dma_start(kvec[:], kvec_d[:])
        for j in range(8):
            nc.sync.dma_start(lutT[:, j, :], lut_d[j * 128:(j + 1) * 128, :])

        def tcol(i):
            return thr[:, i:i + 1]

        for s in [s for _ in range(repeat) for s in range(N_STAGES)]:
            # gathered input, already laid out [p=(hp*64+c), d, n]
            chT = xpool.tile([128, DEPTH, NH], F32, name="chT", tag="chT")
            nc.sync.dma_start(chT[:], xg_d[s])

            # ---- descent on [128=(hp,c) x 1024] ----
            xd = [chT[:, d, :] for d in range(DEPTH)]
            nc.vector.tensor_scalar(b0[:], xd[0], tcol(0), None, ALU.is_gt)
            nc.vector.tensor_copy(b0i[:], b0[:])
            nc.vector.tensor_scalar(tA[:], b0[:], tcol(2), tcol(1), ALU.mult, ALU.add)
            nc.vector.tensor_tensor(b1[:], xd[1], tA[:], ALU.is_gt)
            nc.vector.tensor_copy(b1i[:], b1[:])

            nc.vector.tensor_scalar(tB[:], b1[:], tcol(4), tcol(3), ALU.mult, ALU.add)
            nc.vector.tensor_scalar(tC[:], b1[:], tcol(6), tcol(5), ALU.mult, ALU.add)
            nc.vector.tensor_copy(tA[:], tB[:])
            nc.vector.copy_predicated(tA[:], b0i[:], tC[:])
            nc.vector.tensor_tensor(b2[:], xd[2], tA[:], ALU.is_gt)

            nc.vector.tensor_scalar(tB[:], b2[:], tcol(8), tcol(7), ALU.mult, ALU.add)
            nc.vector.tensor_scalar(tC[:], b2[:], tcol(10), tcol(9), ALU.mult, ALU.add)
            nc.vector.tensor_scalar(tD[:], b2[:], tcol(12), tcol(11), ALU.mult, ALU.add)
            nc.vector.tensor_scalar(tE[:], b2[:], tcol(14), tcol(13), ALU.mult, ALU.add)
            nc.vector.tensor_copy(tF[:], tB[:])
            nc.vector.copy_predicated(tF[:], b1i[:], tC[:])
            nc.vector.tensor_copy(tG[:], tD[:])
            nc.vector.copy_predicated(tG[:], b1i[:], tE[:])
            nc.vector.tensor_copy(tA[:], tF[:])
            nc.vector.copy_predicated(tA[:], b0i[:], tG[:])
            nc.vector.tensor_tensor(tD[:], xd[3], tA[:], ALU.is_gt)   # b3 -> tD

            # bucket = ((b0*2+b1)*2+b2)*2+b3
            bucketbf = bpool.tile([128, NH], BF16, name="bucketbf", tag="bucketbf")
            nc.vector.scalar_tensor_tensor(tB[:], b0[:], 2.0, b1[:], ALU.mult, ALU.add)
            nc.vector.scalar_tensor_tensor(tC[:], tB[:], 2.0, b2[:], ALU.mult, ALU.add)
            nc.vector.scalar_tensor_tensor(
                bucketbf[:], tC[:], 2.0, tD[:], ALU.mult, ALU.add
            )

            # ---- E^T: replicate bucket row of channel 8j+cl to partitions
            # p = k*8 + cl (16 copies via log2 doubling), compare k = p//8 ----
            ET = epool.tile([128, 8, ROWS_STAGE], BF16, name="ET", tag="ET")
            for j in range(8):
                for hp in range(2):
                    fsl = slice(hp * NH, (hp + 1) * NH)
                    nc.scalar.dma_start(
                        ET[0:8, j, fsl],
                        bucketbf[hp * 64 + 8 * j:hp * 64 + 8 * j + 8, :],
                    )
                    for dbl in range(4):
                        w = 8 << dbl
                        nc.scalar.dma_start(
                            ET[w:2 * w, j, fsl], ET[0:w, j, fsl]
                        )
                nc.vector.tensor_scalar(
                    ET[:, j, :], ET[:, j, :], kvec[:], None, ALU.is_equal
                )

            # ---- matmul + output ----
            for i in range(16):
                ps = [
                    pspool.tile([128, 512], F32, name=f"ps{mc}", tag=f"ps{mc}")
                    for mc in range(2)
                ]
                for j in range(8):
                    lhsT = ET[:, j, i * 128:(i + 1) * 128]
                    for mc in range(2):
                        nc.tensor.matmul(
                            ps[mc][:], lhsT, lutT[:, j, mc * 512:(mc + 1) * 512],
                            start=(j == 0), stop=(j == 7),
                        )
                osb = opool.tile([128, M], BF16, name="osb", tag="osb")
                nc.scalar.activation(osb[:, 0:512], ps[0][:], AFT.Copy)
                nc.scalar.activation(osb[:, 512:1024], ps[1][:], AFT.Copy)
                r0 = s * ROWS_STAGE + i * 128
                nc.sync.dma_start(out_d[r0:r0 + 128, :], osb[:])
        es.close()
    nc.finalize()
    return nc


def _prep_inputs(inputMatrix, dims, thresholds, lut):
    x = np.asarray(inputMatrix, dtype=np.float32)
    dims_a = np.asarray(dims).ravel().astype(np.int64)
    thr = np.asarray(thresholds, dtype=np.float32).reshape(C, K - 1)
    lut = np.asarray(lut, dtype=np.float32)

    # thrcols [128, 15]: t0,t1,d21,t3,d43,t5,d65,t7,d87,t9,d109,t11,d1211,t13,d1413
    tcols = np.empty((C, 15), dtype=np.float32)
    tcols[:, 0] = thr[:, 0]
    pairs = [(1, 2), (3, 4), (5, 6), (7, 8), (9, 10), (11, 12), (13, 14)]
    for idx, (lo, hi) in enumerate(pairs):
        tcols[:, 1 + 2 * idx] = thr[:, lo]
        tcols[:, 2 + 2 * idx] = thr[:, hi] - thr[:, lo]
    thrcols = np.concatenate([tcols, tcols], axis=0)  # [128, 15]

    # lutT row p = j*128 + k*8 + cl  (channel = 8j+cl), col m
    lutT = (
        lut.reshape(M, 8, 8, K)           # [M, j, cl, k]
        .transpose(1, 3, 2, 0)            # [j, k, cl, M]
        .reshape(C * K, M)
        .astype(ml_dtypes.bfloat16)
    )

    kvec = (np.arange(128) // 8).astype(np.float32).reshape(128, 1)

    # per-core gathered input: xg[s][hp*64+c][d][n] =
    #   x[core*4096 + s*2048 + hp*1024 + n, dims[4c+d]]
    NH = ROWS_STAGE // 2
    xgs = []
    for i in range(N_CORES):
        g = x[i * N_CORE:(i + 1) * N_CORE][:, dims_a]        # [4096, 256]
        g = g.reshape(N_STAGES, 2, NH, C, DEPTH)             # [s, hp, n, c, d]
        xg = np.ascontiguousarray(g.transpose(0, 1, 3, 4, 2)).reshape(
            N_STAGES, 128, DEPTH, NH
        )
        xgs.append(xg)
    return xgs, thrcols, lutT, kvec


def make_in_maps(prepped):
    xgs, thrcols, lutT, kvec = prepped
    return [
        {"xg": xgs[i], "thrcols": thrcols, "lutT": lutT, "kvec": kvec}
        for i in range(N_CORES)
    ]


def kernel(inputMatrix, dims, thresholds, lut, selection_matrix=None,
           tree_des_mat=None):
    from concourse.bass_utils import run_bass_kernel_spmd

    in_maps = make_in_maps(_prep_inputs(inputMatrix, dims, thresholds, lut))
    nc = build_program()
    res = run_bass_kernel_spmd(nc, in_maps, list(range(N_CORES)))
    out = np.concatenate(
        [np.asarray(res.results[i]["out"]) for i in range(N_CORES)], axis=0
    )
    return out.astype(np.float32)
